# revision 7
# baseline (speedup 1.0000x reference)
"""Trainium2 Bass kernel for nn_Attention_9122510537215 (gnn_message_passing).

Math (per batch b):
    Q = query @ Wq.T + bq                  [LQ=256, 256]
    K = input @ Wk.T + bk                  [LK, 256]
    V = input @ Wv.T + bv                  [LK, 256]
    alpha = softmax_k(Q @ K.T / 16)        [256, LK]
    out[j] = sum_k alpha[j, k] * V[k, j]   [256]

Algebraic restructuring used here:
  * bk shifts every score column by a constant along k -> cancels in softmax_k.
  * G[b] = Wk.T @ (query_b @ Wq.T + bq).T / 16, so scoresT = input @ G  ([LK, 256]).
  * Instead of materializing V, accumulate H[j, i] = sum_k e[k, j] * input[k, i]
    (e = exp(scores)); then numer[j] = sum_i H[j, i] * Wv[j, i] and an appended
    ones-column of the input yields denom[j] = H[j, 256].  bv is applied at the
    end:  out = numer / denom + bv.
  * Softmax is computed unnormalized without max-subtraction (scores are O(1)
    for this problem family; exp stays in [e-4, e+4]).

Distribution: the LK (node) axis is zero-padded to 50176 = 8 * 6272 and sharded
across the 8 NeuronCores; each core returns its partial H accumulators
([B, 2, 128, 257] fp32) and the host reduces across cores in float64.
Padded rows carry a 0 in the ones-column so they contribute nothing.

Device layout: the host pre-casts the input to fp16 and ships both the natural
[k, i] layout (rhs of the H matmul, with ones-column appended) and the
transposed [i, k] layout (stationary operand of the scores matmul), so the
device does no transposes or casts: TensorE streams fp16 matmuls, ScalarE does
exp, VectorE is idle.
"""

import numpy as np
from contextlib import ExitStack

import concourse.bass as bass
import concourse.mybir as mybir
import concourse.tile as tile
from concourse import bacc
from concourse.bass_utils import run_bass_kernel_spmd

# Problem constants (hardcoded; kernel.py must be self-contained).
B = 4
LQ = 256
LK = 50000
OUT = 256
KV = 256            # input feature dim
NORM = 1.0 / 16.0   # 1/sqrt(OUT)

N_CORES = 8
SUB = 128                  # nodes per subtile (PE contraction width)
NSUB = 49                  # subtiles per core per batch
KS = NSUB * SUB            # 6272 nodes per core per batch
LK_PAD = KS * N_CORES      # 50176
SUPER = 4                  # subtiles per supertile (exp batching / PSUM tiling)

F16 = mybir.dt.float16
F32 = mybir.dt.float32


def build(ks=KS, super_=SUPER):
    """Emit the per-core SPMD Bass module (identical on all cores)."""
    nsub = ks // SUB
    groups = [super_] * (nsub // super_)
    if nsub % super_:
        groups.append(nsub % super_)

    nc = bacc.Bacc("TRN2", target_bir_lowering=False, debug=False,
                   num_devices=N_CORES)
    xn = nc.dram_tensor("xn", [B, ks, 258], F16, kind="ExternalInput")
    xt = nc.dram_tensor("xt", [B, 256, ks], F16, kind="ExternalInput")
    g = nc.dram_tensor("g", [B, 256, 256], F16, kind="ExternalInput")
    ht = nc.dram_tensor("ht", [B, 2, 128, 257], F32, kind="ExternalOutput")

    with ExitStack() as ctx:
        tc = ctx.enter_context(tile.TileContext(nc))
        gp = ctx.enter_context(tc.tile_pool(name="gp", bufs=1))
        natp = ctx.enter_context(tc.tile_pool(name="natp", bufs=6))
        tpp = ctx.enter_context(tc.tile_pool(name="tpp", bufs=6))
        ep = ctx.enter_context(tc.tile_pool(name="ep", bufs=3))
        hout = ctx.enter_context(tc.tile_pool(name="hout", bufs=2))
        spp = ctx.enter_context(tc.tile_pool(name="spp", bufs=2, space="PSUM"))
        hpp = ctx.enter_context(tc.tile_pool(name="hpp", bufs=2, space="PSUM"))

        # G for all batches, resident in SBUF: [i(2x128 part), q(256)].
        g_sb = gp.tile([128, B, 2, 256], F16)
        for b in range(B):
            for ih in range(2):
                nc.sync.dma_start(out=g_sb[:, b, ih, :],
                                  in_=g[b, ih * 128:(ih + 1) * 128, :])

        n_it = len(groups)
        for b in range(B):
            ht0 = hpp.tile([128, 257], F32, tag="ht0")
            ht1 = hpp.tile([128, 257], F32, tag="ht1")
            pend = None  # (e, nat, sz, is_first) of the previous supertile
            for t in range(n_it + 1):
                if t < n_it:
                    sz = groups[t]
                    k0 = t * super_ * SUB
                    w = sz * SUB
                    # One DMA writer per tile (multiple writers on different
                    # DMA queues blow the per-instruction sync-wait budget).
                    nats = []
                    for s in range(sz):
                        natt = natp.tile([128, 258], F16, tag=f"nat{s}")
                        eng = nc.sync if s % 2 == 0 else nc.gpsimd
                        eng.dma_start(out=natt[:, :],
                                      in_=xn[b, k0 + s * SUB:k0 + (s + 1) * SUB, :])
                        nats.append(natt)
                    # Transposed layout: [128i, <=256k] chunk tiles.
                    nchunk = (w + 255) // 256
                    tpts = []
                    for ih in range(2):
                        row = []
                        for cc in range(nchunk):
                            c0 = cc * 256
                            cw = min(256, w - c0)
                            tptt = tpp.tile([128, 256], F16, tag=f"tp{ih}{cc}")
                            eng = nc.gpsimd if (ih + cc) % 2 == 0 else nc.sync
                            eng.dma_start(
                                out=tptt[:, :cw],
                                in_=xt[b, ih * 128:(ih + 1) * 128,
                                       k0 + c0:k0 + c0 + cw])
                            row.append(tptt)
                        tpts.append(row)
                    # scoresT[k, q] = sum_i inpT[i, k].T @ G[i, q]
                    sp = spp.tile([128, super_, 256], F32)
                    for s in range(sz):
                        cc, off = divmod(s * SUB, 256)
                        nc.tensor.matmul(sp[:, s, :],
                                         tpts[0][cc][:, off:off + SUB],
                                         g_sb[:, b, 0, :],
                                         start=True, stop=False)
                        nc.tensor.matmul(sp[:, s, :],
                                         tpts[1][cc][:, off:off + SUB],
                                         g_sb[:, b, 1, :],
                                         start=False, stop=True)
                    e = ep.tile([128, super_, 256], F16)
                    nc.scalar.activation(e[:, :sz, :], sp[:, :sz, :],
                                         mybir.ActivationFunctionType.Exp)
                # H matmuls of the previous supertile (keeps PE busy while
                # ScalarE computes this supertile's exp).
                if pend is not None:
                    pe_, pnats, psz, pfirst = pend
                    for s in range(psz):
                        is_first = pfirst and s == 0
                        is_last = (t == n_it) and s == psz - 1
                        nc.tensor.matmul(ht0[:, :], pe_[:, s, 0:128],
                                         pnats[s][:, 0:257],
                                         start=is_first, stop=is_last)
                        nc.tensor.matmul(ht1[:, :], pe_[:, s, 128:256],
                                         pnats[s][:, 0:257],
                                         start=is_first, stop=is_last)
                if t < n_it:
                    pend = (e, nats, sz, t == 0)
            hts = hout.tile([128, 2, 257], F32)
            nc.vector.tensor_copy(hts[:, 0, :], ht0[:, :])
            nc.vector.tensor_copy(hts[:, 1, :], ht1[:, :])
            nc.sync.dma_start(out=ht[b, 0], in_=hts[:, 0, :])
            nc.sync.dma_start(out=ht[b, 1], in_=hts[:, 1, :])
    nc.compile()
    return nc


def _prepare_inputs(query, input, Wq, bq, Wk):
    """Host-side marshalling: G matrices + fp16 input in both layouts, sharded."""
    # G[b] = Wk.T @ (query_b @ Wq.T + bq).T * NORM   -> [B, 256(i), 256(q)]
    Q = query.astype(np.float64) @ Wq.T.astype(np.float64) + bq
    G = np.einsum('di,bqd->biq', Wk.astype(np.float64), Q) * NORM
    g16 = np.ascontiguousarray(G.astype(np.float32).astype(np.float16))

    xn = np.zeros((B, LK_PAD, 258), np.float16)
    xn[:, :LK, :256] = input.astype(np.float16)
    xn[:, :LK, 256] = 1.0   # ones-column -> denom; stays 0 on padded rows
    xt_view = xn[:, :, :256].transpose(0, 2, 1)  # [B, 256, LK_PAD] view

    in_maps = []
    for c in range(N_CORES):
        sl = slice(c * KS, (c + 1) * KS)
        in_maps.append({
            "xn": np.ascontiguousarray(xn[:, sl, :]),
            "xt": np.ascontiguousarray(xt_view[:, :, sl]),
            "g": g16,
        })
    return in_maps


def kernel(query, input, Wq, bq, Wk, bk, Wv, bv):
    # bk provably cancels in softmax over k; bq is folded into G; bv is applied
    # in the host-side epilogue below.
    query = np.asarray(query, dtype=np.float32)
    input = np.asarray(input, dtype=np.float32)
    Wq = np.asarray(Wq, dtype=np.float32)
    bq = np.asarray(bq, dtype=np.float32)
    Wk = np.asarray(Wk, dtype=np.float32)
    Wv = np.asarray(Wv, dtype=np.float32)
    bv = np.asarray(bv, dtype=np.float32)

    nc = build()
    in_maps = _prepare_inputs(query, input, Wq, bq, Wk)
    res = run_bass_kernel_spmd(nc, in_maps, core_ids=list(range(N_CORES)))
    kernel._last_result = res

    numer = np.zeros((B, OUT))
    denom = np.zeros((B, OUT))
    Wv64 = Wv.astype(np.float64)
    for r in res.results:
        H = r["ht"].astype(np.float64).reshape(B, OUT, 257)  # j = half*128 + p
        numer += (H[:, :, :256] * Wv64[None]).sum(axis=2)
        denom += H[:, :, 256]
    out = numer / denom + bv
    return out.astype(np.float32)


if __name__ == "__main__":
    # CoreSim smoke test on a reduced size (5 subtiles -> groups [4, 1]).
    from concourse.bass_interp import CoreSim

    ks = 5 * SUB
    rng = np.random.default_rng(0)
    xn_np = rng.standard_normal((B, ks, 258)).astype(np.float16)
    xn_np[:, :, 256] = 1.0
    xn_np[:, :, 257] = 0.0
    xt_np = np.ascontiguousarray(xn_np[:, :, :256].transpose(0, 2, 1))
    g_np = (rng.standard_normal((B, 256, 256)) * 0.05).astype(np.float16)

    nc = build(ks=ks)
    sim = CoreSim(nc)
    sim.tensor("xn")[:] = xn_np
    sim.tensor("xt")[:] = xt_np
    sim.tensor("g")[:] = g_np
    sim.simulate()
    got = np.array(sim.tensor("ht")).reshape(B, OUT, 257)

    x = xn_np[:, :, :257].astype(np.float32)
    want = np.zeros((B, OUT, 257), np.float32)
    for b in range(B):
        s = x[b, :, :256] @ g_np[b].astype(np.float32)
        e = np.exp(s).astype(np.float16).astype(np.float32)
        want[b] = e.T @ x[b]
    err = np.abs(got - want).max() / np.abs(want).max()
    print("CoreSim rel err:", err)
    assert err < 2e-2, err
    print("OK")
